# revision 1
# baseline (speedup 1.0000x reference)
"""GPT-NeoX attention block on 8 Trainium2 NeuronCores (Bass/Tile).

Sharding: tensor-parallel over heads (16 heads -> 2 per core). Each core:
  - projects its 2 heads' q,k (feature-major) and v (token-major) from the
    full hidden states,
  - applies partial RoPE (rotary_dim=32) to q,k,
  - computes causal attention for its heads (scores kept k-major so softmax
    sums run on the PE via ones-matmuls; no transposes needed),
  - AllToAll redistributes attention outputs from head-sharded to
    token-sharded,
  - computes its 512-token slice of the output projection.
Host concatenates the 8 token slices and adds the (bias) correction term.

All heavy matmuls run as float32r (TF32-like) for 4x PE throughput; set
MM_F32R = False to fall back to exact fp32 matmuls.
"""
import sys

sys.path.insert(0, "/opt/trn_rl_repo")

import numpy as np

import concourse.bass as bass
import concourse.tile as tile
from concourse import bacc, mybir

# ---------------------------------------------------------------- constants
NUM_HEADS = 16
HIDDEN = 2048
HEAD_DIM = 128
ROTARY_DIM = 32
ROPE_BASE = 10000.0
B, S = 2, 2048
T = B * S                      # 4096 tokens
NCORES = 8
HPC = NUM_HEADS // NCORES      # 2 heads per core
W1 = 256                       # phase-1 token-chunk width
NQB = S // 128                 # 16 q/k blocks per batch
import os
MM_F32R = os.environ.get('MM_F32R', '1') == '1'  # float32r matmuls (4x faster, ~1e-3 rel err)
NEG_BIG = -30000.0             # causal-mask additive constant (exp -> 0)

f32 = mybir.dt.float32
f32r = mybir.dt.float32r
MM_DT = f32r if MM_F32R else f32

_PROGRAM_CACHE = {}


def _mm_cast(ap):
    return ap.bitcast(f32r) if MM_F32R else ap


def _build_program():
    """Build the SPMD Bass program (identical on all 8 cores)."""
    nc = bacc.Bacc(num_devices=NCORES, dynamic_dma_scratch_size=4096)

    xT = nc.dram_tensor("xT", [HIDDEN, T], f32, kind="ExternalInput")
    wq = nc.dram_tensor("wq", [HIDDEN, HPC * HEAD_DIM], f32, kind="ExternalInput")
    wk = nc.dram_tensor("wk", [HIDDEN, HPC * HEAD_DIM], f32, kind="ExternalInput")
    wv = nc.dram_tensor("wv", [HIDDEN, HPC * HEAD_DIM], f32, kind="ExternalInput")
    wout = nc.dram_tensor("wout", [HIDDEN, HIDDEN], f32, kind="ExternalInput")
    cosd = nc.dram_tensor("cosd", [ROTARY_DIM, T], f32, kind="ExternalInput")
    sind = nc.dram_tensor("sind", [ROTARY_DIM, T], f32, kind="ExternalInput")
    trid = nc.dram_tensor("trid", [128, 128], f32, kind="ExternalInput")
    sgnd = nc.dram_tensor("sgnd", [ROTARY_DIM, 1], f32, kind="ExternalInput")
    onekd = nc.dram_tensor("onekd", [128, 1], f32, kind="ExternalInput")
    onebd = nc.dram_tensor("onebd", [1, 128], f32, kind="ExternalInput")
    out = nc.dram_tensor("out", [T // NCORES, HIDDEN], f32, kind="ExternalOutput")

    KC = HIDDEN // 128          # 16 contraction chunks
    NCH = T // W1               # 16 phase-1 token chunks
    shuffle_mask = [(i + 16) % 32 for i in range(32)]

    with tile.TileContext(nc) as tc:
        import contextlib

        with contextlib.ExitStack() as ctx:
            persist = ctx.enter_context(tc.tile_pool(name="persist", bufs=1))
            dram = ctx.enter_context(tc.tile_pool(name="dram", bufs=1, space="DRAM"))
            p12 = ctx.enter_context(contextlib.ExitStack())
            qkvpool = p12.enter_context(tc.tile_pool(name="qkvpool", bufs=1))

            qT = qkvpool.tile([128, HPC, T], MM_DT, name="qT", tag="qT")
            kT = qkvpool.tile([128, HPC, T], MM_DT, name="kT", tag="kT")
            # token-major V: [tp, tt, c]; t = tt*128+tp, c = head*128+d
            vtm = qkvpool.tile([128, T // 128, HPC * HEAD_DIM], MM_DT, name="vtm", tag="vtm")
            tri = persist.tile([128, 128], f32, name="tri", tag="tri")
            sgn = persist.tile([32, 1], f32, name="sgn", tag="sgn")
            ones_k = persist.tile([128, 1], MM_DT, name="ones_k", tag="ones_k")

            nc.sync.dma_start(out=tri[:], in_=trid[:])
            nc.sync.dma_start(out=sgn[:], in_=sgnd[:])
            nc.sync.dma_start(out=ones_k[:], in_=_mm_cast(onekd[:]))

            a2a_in0 = dram.tile([NCORES, HPC * HEAD_DIM, 256], f32, name="a2a_in0", tag="a2a_in0")
            a2a_in1 = dram.tile([NCORES, HPC * HEAD_DIM, 256], f32, name="a2a_in1", tag="a2a_in1")
            a2a_out0 = dram.tile([NCORES, HPC * HEAD_DIM, 256], f32, name="a2a_out0", tag="a2a_out0")
            a2a_out1 = dram.tile([NCORES, HPC * HEAD_DIM, 256], f32, name="a2a_out1", tag="a2a_out1")

            # ---------------------------------------------- phase 1: qkv
            with contextlib.ExitStack() as p1:
                wpool = p1.enter_context(tc.tile_pool(name="wpool", bufs=1))
                xpool = p1.enter_context(tc.tile_pool(name="xpool", bufs=2))
                rpool = p1.enter_context(tc.tile_pool(name="rpool", bufs=4))
                ps_qk = p1.enter_context(tc.tile_pool(name="ps_qk", bufs=4, space="PSUM"))
                ps_v = p1.enter_context(tc.tile_pool(name="ps_v", bufs=3, space="PSUM"))

                wq_sb = wpool.tile([128, KC, HPC * HEAD_DIM], MM_DT, name="wq_sb", tag="wq_sb")
                wk_sb = wpool.tile([128, KC, HPC * HEAD_DIM], MM_DT, name="wk_sb", tag="wk_sb")
                wv_sb = wpool.tile([128, KC, HPC * HEAD_DIM], MM_DT, name="wv_sb", tag="wv_sb")
                cos_sb = wpool.tile([ROTARY_DIM, T], f32, name="cos_sb", tag="cos_sb")
                sin_sb = wpool.tile([ROTARY_DIM, T], f32, name="sin_sb", tag="sin_sb")

                xT_r = xT[:].rearrange("(kc kp) t -> kp kc t", kp=128)

                # DMA issue order = model scheduling order: first q weights and
                # the first x chunk (gates the first matmul group), then the
                # rest of the weights/tables.
                wq_r = wq[:].rearrange("(kc kp) c -> kp kc c", kp=128)
                xn0 = xpool.tile([128, KC, W1], MM_DT, name="xn0", tag="xn")
                for g in range(2):
                    kcs = slice(8 * g, 8 * (g + 1))
                    nc.sync.dma_start(out=wq_sb[:, kcs, :], in_=_mm_cast(wq_r[:, kcs, :]))
                    nc.sync.dma_start(out=xn0[:, kcs, :], in_=_mm_cast(xT_r[:, kcs, 0:W1]))
                for w_sb, w_dram in ((wk_sb, wk), (wv_sb, wv)):
                    wr = w_dram[:].rearrange("(kc kp) c -> kp kc c", kp=128)
                    nc.sync.dma_start(out=w_sb[:], in_=_mm_cast(wr))
                nc.sync.dma_start(out=cos_sb[:], in_=cosd[:])
                nc.sync.dma_start(out=sin_sb[:], in_=sind[:])

                for n in range(NCH):
                    tcol = slice(n * W1, (n + 1) * W1)
                    if n == 0:
                        xn = xn0
                    else:
                        xn = xpool.tile([128, KC, W1], MM_DT, name=f"xn{n}", tag="xn")
                        nc.sync.dma_start(out=xn[:], in_=_mm_cast(xT_r[:, :, tcol]))

                    # q/k feature-major: psum[c, t] += w[k, c].T @ x[k, t]
                    for ct in range(4):
                        w_sb = wq_sb if ct < 2 else wk_sb
                        h = ct % 2
                        tgt = qT if ct < 2 else kT
                        pqk = ps_qk.tile([128, W1], f32, name=f"pqk{n}_{ct}", tag="pqk")
                        for kc in range(KC):
                            nc.tensor.matmul(
                                pqk[:],
                                w_sb[:, kc, h * 128:(h + 1) * 128],
                                xn[:, kc, :],
                                start=(kc == 0),
                                stop=(kc == KC - 1),
                            )
                        nc.scalar.copy(out=tgt[:, h, tcol], in_=pqk[:])

                    # v token-major: psum[t, c] += x[k, t].T @ wv[k, c]
                    for t2 in range(W1 // 128):
                        pv = ps_v.tile([128, HPC * HEAD_DIM], f32, name=f"pv{n}_{t2}", tag="pv")
                        for kc in range(KC):
                            nc.tensor.matmul(
                                pv[:],
                                xn[:, kc, t2 * 128:(t2 + 1) * 128],
                                wv_sb[:, kc, :],
                                start=(kc == 0),
                                stop=(kc == KC - 1),
                            )
                        nc.scalar.copy(out=vtm[:, n * (W1 // 128) + t2, :], in_=pv[:])

                    # RoPE on the rotary rows of this chunk, once 2 chunks ready
                    if n % 2 == 1:
                        seg = slice((n - 1) * W1, (n + 1) * W1)
                        for tgt in (qT, kT):
                            for h in range(HPC):
                                shuf = rpool.tile([32, 2 * W1], f32, name=f"shuf{n}_{h}", tag="shuf")
                                nc.vector.stream_shuffle(shuf[:], tgt[0:32, h, seg], shuffle_mask)
                                nc.vector.scalar_tensor_tensor(
                                    out=shuf[:],
                                    in0=shuf[:],
                                    scalar=sgn[:, 0:1],
                                    in1=sin_sb[:, seg],
                                    op0=mybir.AluOpType.mult,
                                    op1=mybir.AluOpType.mult,
                                )
                                nc.vector.tensor_mul(tgt[0:32, h, seg], tgt[0:32, h, seg], cos_sb[:, seg])
                                nc.vector.tensor_add(tgt[0:32, h, seg], tgt[0:32, h, seg], shuf[:])

            # ---------------------------------------------- phase 2: attention
            with contextlib.ExitStack() as p2:
                apool = p2.enter_context(tc.tile_pool(name="apool", bufs=6))
                ptpool = p2.enter_context(tc.tile_pool(name="ptpool", bufs=6))
                ps_s = p2.enter_context(tc.tile_pool(name="ps_s", bufs=4, space="PSUM"))
                ps_pv = p2.enter_context(tc.tile_pool(name="ps_pv", bufs=3, space="PSUM"))
                ps_l = p2.enter_context(tc.tile_pool(name="ps_l", bufs=1, space="PSUM"))

                # prefetch half of w_out into the right SBUF edge while
                # attention runs (left edge is pinned by qT/kT/vtm until the
                # last attention matmul); separate side => separate LIFO stack
                woE = ctx.enter_context(tc.tile_pool(name="woE", bufs=1, side="right"))
                wo_early = []
                for dc in range(8):
                    wt = woE.tile([128, HIDDEN], MM_DT, name=f"wo{dc}", tag=f"wo{dc}")
                    nc.sync.dma_start(out=wt[:], in_=_mm_cast(wout[dc * 128:(dc + 1) * 128, :]))
                    wo_early.append(wt)

                attn_insts = []
                # q-chunks of 256 tokens; parity 0 chunks (first half of every
                # core's token slice) first, so AllToAll #0 overlaps the
                # parity-1 attention compute.
                for parity in range(2):
                    for b in range(B):
                        for h in range(HPC):
                            for c2 in range(parity, 8, 2):   # 256-wide chunks in batch b
                                nkb = 2 * c2 + 2
                                qcol = slice(b * S + c2 * 256, b * S + (c2 + 1) * 256)
                                ppv = ps_pv.tile([128, 256], f32, name=f"ppv{b}{h}{c2}", tag="ppv")
                                pl = ps_l.tile([1, 256], f32, name=f"pl{b}{h}{c2}", tag="pl")
                                for pair in range(nkb // 2):
                                    # two k-blocks share one psum bank / one exp
                                    ps = ps_s.tile([128, 512], f32, name=f"ps{b}{h}{c2}{pair}", tag="ps")
                                    pt = ptpool.tile([128, 512], MM_DT, name=f"pt{b}{h}{c2}{pair}", tag="pt")
                                    for j in range(2):
                                        kb = 2 * pair + j
                                        kcol = slice(b * S + kb * 128, b * S + (kb + 1) * 128)
                                        nc.tensor.matmul(
                                            ps[:, 256 * j:256 * (j + 1)],
                                            kT[:, h, kcol], qT[:, h, qcol],
                                            start=True, stop=True,
                                        )
                                        p = kb - 2 * c2
                                        if p >= 0:
                                            nc.vector.tensor_add(
                                                ps[:, 256 * j + p * 128:256 * j + (p + 1) * 128],
                                                ps[:, 256 * j + p * 128:256 * j + (p + 1) * 128],
                                                tri[:],
                                            )
                                            if p > 0:
                                                nc.vector.tensor_scalar_add(
                                                    ps[:, 256 * j:256 * j + 128],
                                                    ps[:, 256 * j:256 * j + 128],
                                                    NEG_BIG,
                                                )
                                    nc.scalar.activation(
                                        out=pt[:], in_=ps[:],
                                        func=mybir.ActivationFunctionType.Exp,
                                    )
                                    for j in range(2):
                                        kb = 2 * pair + j
                                        nc.tensor.matmul(
                                            ppv[:],
                                            vtm[:, b * NQB + kb, h * 128:(h + 1) * 128],
                                            pt[:, 256 * j:256 * (j + 1)],
                                            start=(kb == 0), stop=(kb == nkb - 1),
                                            skip_group_check=True,
                                        )
                                    # l partials: both half-blocks accumulate
                                    # into the same [1,256] psum region
                                    for j in range(2):
                                        kb = 2 * pair + j
                                        nc.tensor.matmul(
                                            pl[:], ones_k[:], pt[:, 256 * j:256 * (j + 1)],
                                            start=(kb == 0), stop=(kb == nkb - 1),
                                            skip_group_check=True,
                                        )
                                l_sb = apool.tile([1, 256], MM_DT, name=f"l{b}{h}{c2}", tag="l_sb")
                                nc.scalar.copy(out=l_sb[:], in_=pl[:])
                                lbc = apool.tile([128, 256], f32, name=f"lbc{b}{h}{c2}", tag="lbc")
                                nc.gpsimd.partition_broadcast(lbc[:], l_sb[:].bitcast(f32))
                                recip = apool.tile([128, 256], f32, name=f"rc{b}{h}{c2}", tag="recip")
                                nc.vector.reciprocal(out=recip[:], in_=lbc[:])
                                attn_sb = apool.tile([128, 256], f32, name=f"at{b}{h}{c2}", tag="attn_sb")
                                attn_insts.append(nc.vector.tensor_mul(attn_sb[:], ppv[:], recip[:]))
                                a2a_buf = a2a_in0 if parity == 0 else a2a_in1
                                nc.sync.dma_start(
                                    out=a2a_buf[b * 4 + c2 // 2, h * 128:(h + 1) * 128, :],
                                    in_=attn_sb[:],
                                )
                    if parity == 0:
                        nc.gpsimd.collective_compute(
                            "AllToAll",
                            mybir.AluOpType.bypass,
                            replica_groups=[list(range(NCORES))],
                            ins=[a2a_in0.opt()],
                            outs=[a2a_out0.opt()],
                        )
                nc.gpsimd.collective_compute(
                    "AllToAll",
                    mybir.AluOpType.bypass,
                    replica_groups=[list(range(NCORES))],
                    ins=[a2a_in1.opt()],
                    outs=[a2a_out1.opt()],
                )

            wo_sb = []
            N_PREFETCH = 0
            p12.close()   # release qT/kT/vtm SBUF before the out-proj pools open

            # ---------------------------------------------- phase 3: out-proj
            with contextlib.ExitStack() as p3:
                wopoolB = p3.enter_context(tc.tile_pool(name="wopoolB", bufs=1))
                opool = p3.enter_context(tc.tile_pool(name="opool", bufs=4))
                ps_o = p3.enter_context(tc.tile_pool(name="ps_o", bufs=1, space="PSUM"))

                wo_sb.extend(wo_early)
                for dc in range(8, KC):
                    wt = wopoolB.tile([128, HIDDEN], MM_DT, name=f"wo{dc}", tag=f"wo{dc}")
                    nc.sync.dma_start(out=wt[:], in_=_mm_cast(wout[dc * 128:(dc + 1) * 128, :]))
                    wo_sb.append(wt)

                for half, a2a_o in ((0, a2a_out0), (1, a2a_out1)):
                    # attnT for my 256 tokens of this half: [dp, dc, t]
                    attnT = wopoolB.tile([128, KC, 256], MM_DT, name=f"attnT{half}", tag="attnT", bufs=2)
                    nc.sync.dma_start(
                        out=attnT[:],
                        in_=_mm_cast(
                            a2a_o[:]
                            .rearrange("s q t -> (s q) t")
                            .rearrange("(dc dp) t -> dp dc t", dp=128)
                        ),
                    )
                    # dc-outer accumulation: each arriving wout chunk is
                    # consumed immediately; 8 psum banks live (2 tt x 4 oc)
                    pos = [
                        ps_o.tile([128, 512], f32, name=f"po{half}{tt2}{oc}", tag=f"po{tt2}{oc}")
                        for tt2 in range(2) for oc in range(4)
                    ]
                    for dc in range(KC):
                        for tt2 in range(2):
                            for oc in range(4):
                                nc.tensor.matmul(
                                    pos[tt2 * 4 + oc][:],
                                    attnT[:, dc, tt2 * 128:(tt2 + 1) * 128],
                                    wo_sb[dc][:, oc * 512:(oc + 1) * 512],
                                    start=(dc == 0),
                                    stop=(dc == KC - 1),
                                )
                    for tt2 in range(2):
                        tt = half * 2 + tt2
                        osb = opool.tile([128, HIDDEN], f32, name=f"osb{tt}", tag="osb")
                        for oc in range(4):
                            nc.scalar.copy(
                                out=osb[:, oc * 512:(oc + 1) * 512],
                                in_=pos[tt2 * 4 + oc][:],
                            )
                            nc.sync.dma_start(
                                out=out[tt * 128:(tt + 1) * 128, oc * 512:(oc + 1) * 512],
                                in_=osb[:, oc * 512:(oc + 1) * 512],
                            )

    nc.finalize()
    return nc




def _runner():
    """Build (once) a reusable jitted SPMD executor over the 8 cores.

    Returns a callable: in_maps (list of per-core dicts) -> full [T, H] output.
    """
    if "runner" in _PROGRAM_CACHE:
        return _PROGRAM_CACHE["runner"]

    import jax
    from jax.sharding import Mesh, PartitionSpec
    try:
        from jax.experimental.shard_map import shard_map
    except Exception:
        from jax.shard_map import shard_map  # newer jax
    from concourse import bass2jax
    from concourse.bass2jax import _bass_exec_p, partition_id_tensor, install_neuronx_cc_hook

    install_neuronx_cc_hook()
    nc = _build_program()
    _PROGRAM_CACHE["nc"] = nc

    partition_name = nc.partition_id_tensor.name if nc.partition_id_tensor else None
    in_names, out_names, out_avals, zero_outs = [], [], [], []
    for alloc in nc.m.functions[0].allocations:
        if not isinstance(alloc, mybir.MemoryLocationSet):
            continue
        name = alloc.memorylocations[0].name
        if alloc.kind == "ExternalInput":
            if name != partition_name:
                in_names.append(name)
        elif alloc.kind == "ExternalOutput":
            out_names.append(name)
            shape = tuple(alloc.tensor_shape)
            dtype = mybir.dt.np(alloc.dtype)
            out_avals.append(jax.core.ShapedArray(shape, dtype))
            zero_outs.append(np.zeros(shape, dtype))
    n_params = len(in_names)
    all_in_names = list(in_names) + list(out_names)
    if partition_name is not None:
        all_in_names.append(partition_name)

    def _body(*args):
        operands = list(args)
        if partition_name is not None:
            operands.append(partition_id_tensor())
        outs = _bass_exec_p.bind(
            *operands,
            out_avals=tuple(out_avals),
            in_names=tuple(all_in_names),
            out_names=tuple(out_names),
            lowering_input_output_aliases=(),
            sim_require_finite=True,
            sim_require_nnan=True,
            nc=nc,
        )
        return tuple(outs)

    devices = jax.devices()[:NCORES]
    mesh = Mesh(np.asarray(devices), ("core",))
    n_outs = len(out_names)
    sharded = jax.jit(
        shard_map(
            _body,
            mesh=mesh,
            in_specs=(PartitionSpec("core"),) * (n_params + n_outs),
            out_specs=(PartitionSpec("core"),) * n_outs,
            check_rep=False,
        ),
        keep_unused=True,
    )
    concat_zeros = [
        np.zeros((NCORES * z.shape[0], *z.shape[1:]), z.dtype) for z in zero_outs
    ]

    def run(in_maps):
        concat_in = [
            np.concatenate([np.asarray(in_maps[c][nm]) for c in range(NCORES)], axis=0)
            for nm in in_names
        ]
        out_arrs = sharded(*concat_in, *concat_zeros)
        # single output "out": per-core [512, H] concat on axis 0 == full [T, H]
        return np.asarray(out_arrs[out_names.index("out")])

    _PROGRAM_CACHE["runner"] = run
    _PROGRAM_CACHE["runner_parts"] = (sharded, in_names, out_names, concat_zeros, mesh)
    return run

def _rope_tables():
    inv_freq = 1.0 / (ROPE_BASE ** (np.arange(0, ROTARY_DIM, 2, dtype=np.float64) / ROTARY_DIM))
    t = np.arange(S, dtype=np.float64)
    freqs = np.einsum("s,d->sd", t, inv_freq)          # [S, 16]
    emb = np.concatenate([freqs, freqs], axis=-1)       # [S, 32]
    cos = np.cos(emb).T.astype(np.float32)              # [32, S]
    sin = np.sin(emb).T.astype(np.float32)
    cosT = np.tile(cos, (1, B))                         # [32, T]  (batch-tiled)
    sinT = np.tile(sin, (1, B))
    return np.ascontiguousarray(cosT), np.ascontiguousarray(sinT)


def kernel(hidden_states, w_qkv, b_qkv, w_out, b_out):
    hidden_states = np.asarray(hidden_states, dtype=np.float32)
    w_qkv = np.asarray(w_qkv, dtype=np.float32)
    b_qkv = np.asarray(b_qkv, dtype=np.float32)
    w_out = np.asarray(w_out, dtype=np.float32)
    b_out = np.asarray(b_out, dtype=np.float32)


    xT = np.ascontiguousarray(hidden_states.reshape(T, HIDDEN).T)   # [H, T]
    cosT, sinT = _rope_tables()
    # additive causal mask in [k, q] orientation: valid where q >= k
    r = np.arange(128)
    trim = np.where(r[None, :] >= r[:, None], 0.0, NEG_BIG).astype(np.float32)
    sgn_host = np.concatenate([-np.ones(16, np.float32), np.ones(16, np.float32)]).reshape(ROTARY_DIM, 1)
    wout_c = np.ascontiguousarray(w_out)

    in_maps = []
    for core in range(NCORES):
        hs = [HPC * core + j for j in range(HPC)]
        wq_i = np.concatenate([w_qkv[:, h * 384:h * 384 + 128] for h in hs], axis=1)
        wk_i = np.concatenate([w_qkv[:, h * 384 + 128:h * 384 + 256] for h in hs], axis=1)
        wv_i = np.concatenate([w_qkv[:, h * 384 + 256:h * 384 + 384] for h in hs], axis=1)
        in_maps.append({
            "xT": xT,
            "sgnd": sgn_host,
            "onekd": np.ones((128, 1), np.float32),
            "onebd": np.ones((1, 128), np.float32),
            "wq": np.ascontiguousarray(wq_i),
            "wk": np.ascontiguousarray(wk_i),
            "wv": np.ascontiguousarray(wv_i),
            "wout": wout_c,
            "cosd": cosT,
            "sind": sinT,
            "trid": trim,
        })

    out_full = _runner()(in_maps)

    # exact host-side correction for the biases the device ignores:
    # v-bias contributes (softmax rows sum to 1): b_v @ w_out ; plus b_out.
    b_v = np.concatenate([b_qkv[h * 384 + 256:h * 384 + 384] for h in range(NUM_HEADS)])
    corr = b_v.astype(np.float64) @ w_out.astype(np.float64) + b_out.astype(np.float64)
    out_full = out_full + corr.astype(np.float32)[None, :]

    return out_full.reshape(B, S, HIDDEN)

